# revision 74
# baseline (speedup 1.0000x reference)
"""Causal self-attention Trainium2 kernel (8 NeuronCores).

Reference computation (fp32):
    qkv = x @ W_qkv; q,k,v = split(qkv)
    per head: scores = q k^T / sqrt(64), causal softmax, out = attn @ v
    y = out @ W_out

Sharding: 8 cores = 2 batches x 4 head-groups. Core c handles batch
b = c // 4 and heads [4*hg, 4*hg+4) with hg = c % 4. Each core computes
a partial y^T (its 4 heads' contribution through W_out rows); the host
sums the 4 partials per batch.

Fully software-pipelined single schedule (185149 -> 156566 ns in the
TimelineSim cost model): transposes/V-proj/QK-proj groups, attention
jobs and out-projection chunks are interleaved in one PE instruction
stream so the PE never drains between phases; everything is paced by
just-in-time DMA arrival at the front and drains through two DMA issue
paths (HWDGE + Pool soft-DGE) at the tail.

Dataflow per core (projection matmuls fp32r ~= TF32; x, Q^T/K^T, V and
attention weights bf16; PSUM accumulation fp32):
  A. x (bf16, host-cast) -> PE-transpose -> xT [c, t] upconverted to
     f32r on evacuation; 4 transposes per PSUM bank. (fp32 transposes
     cost 2cy/row and f32r transposes fail neuronxcc codegen.)
  B. Qt/Kt = (W_qk^T x^T) directly in [channel, t] layout, bf16.
  C. V natural [t, channel] bf16; ones column at 64 per head (softmax
     denominator accumulates in the AV matmul's row 64 for free).
  D. per (head, q-chunk of 512): S^T blocks = Kt_blk^T Qt_chunk (K=64),
     P = exp(S/8) with the above-diagonal 128-wide square zeroed by a
     Pool affine_select on P; the two smallest diagonal blocks (r2,r3)
     share one PSUM bank and one exp instruction (the ~185ns per-exp
     SBUF access overhead is what paces the attention phase).
     O_aug = V_aug^T P accumulated over s-blocks. Normalize: DVE
     reciprocal of row 64, broadcast across partitions via a K=1 PE
     matmul against a ones column (engines cannot read partition-
     stride-0; DVE cannot read two PSUM operands), DVE row-mul.
     Odd heads DMA-shift rows to partitions 64..127. The normalize and
     each job's last AHEAD AV matmuls are deferred past the next job's
     filler work so PE never waits on the exp/reciprocal chains.
  E. yT[c_out, t] = W_out_slice^T @ attn_outT (K=128 over 2 blocks),
     spread through D as PE filler; y leaves as bf16 (the host
     upconverts and sums partials in fp32). The final q-chunk's
     normalize is split into column halves so its out-projection waves
     overlap the second half's normalize chain.

Scores are O(1) (x ~ N(0,1), W scaled 1/sqrt(1024)), |s| < ~8, so
softmax max-subtraction is skipped; exp is computed directly. Masked
positions exp to finite garbage and are zeroed by the affine_select.

This container's walrus accepts at most ONE on_wait per instruction while
Tile emits several; split_multi_waits() legalizes the program after
TileContext exit.
"""

import math
from contextlib import ExitStack

import numpy as np

import concourse.bass as bass
import concourse.mybir as mybir
import concourse.tile as tile
from concourse.bass_utils import run_bass_kernel_spmd
from concourse.masks import make_identity

F32 = mybir.dt.float32
F32R = mybir.dt.float32r
BF16 = mybir.dt.bfloat16

B, T, C = 2, 2048, 1024
N_HEADS, HEAD_DIM = 16, 64
HEADS_PER_CORE = 4          # 4 heads/core (16 heads / 4 head-groups)
HC = HEADS_PER_CORE * HEAD_DIM  # 256 channels per core
N_CORES = 8
TB = T // 128               # 16 t-blocks of 128
QC = T // 512               # 4 q-chunks of 512
CB = C // 128               # 8 c_in blocks


def split_multi_waits(nc):
    """Walrus here allows only one on_wait per instruction; move extras to
    standalone EventSemaphore instructions on the same engine."""
    n_split = 0
    for fn in nc.m.functions:
        for bb in fn.blocks:
            if not any(
                inst.sync_info is not None and len(inst.sync_info.on_wait) > 1
                for inst in bb.instructions
            ):
                continue
            out = []
            for inst in bb.instructions:
                si = inst.sync_info
                if si is not None and len(si.on_wait) > 1:
                    waits = list(si.on_wait)
                    for i, w in enumerate(waits[:-1]):
                        out.append(
                            mybir.InstEventSemaphore(
                                name=f"{inst.name}_sw{i}",
                                engine=inst.engine,
                                sync_info=mybir.SyncInfo(on_wait=[w], on_update=[]),
                            )
                        )
                        n_split += 1
                    inst.sync_info = mybir.SyncInfo(
                        on_wait=[waits[-1]], on_update=list(si.on_update)
                    )
                out.append(inst)
            bb.instructions = out
    return n_split


def build():
    nc = bass.Bass(trn_type="TRN2")
    # x arrives as bf16 (host-cast): halves the front-critical x DMA bytes
    # and makes the PE transposes 1.0 cy/row (fp32 is 2.0; f32r transposes
    # fail neuronxcc codegen). xT is upconverted to f32r on evacuation, so
    # all downstream matmuls stay fp32r.
    xb = nc.dram_tensor("xb", [T, C], BF16, kind="ExternalInput")
    wqk = nc.dram_tensor("wqk", [C, 2 * HC], F32R, kind="ExternalInput")
    wv = nc.dram_tensor("wv", [C, HC], F32R, kind="ExternalInput")
    wo = nc.dram_tensor("wo", [HC, C], F32R, kind="ExternalInput")
    # y partials leave the core as bf16 (halves the trailing output-DMA
    # serialization); the host upconverts and sums partials in fp32
    yt = nc.dram_tensor("yt", [C, T], BF16, kind="ExternalOutput")

    scale = 1.0 / math.sqrt(HEAD_DIM)

    with tile.TileContext(nc) as tc, ExitStack() as ctx:
        glob = ctx.enter_context(tc.tile_pool(name="glob", bufs=1))
        xstage = ctx.enter_context(tc.tile_pool(name="xstage", bufs=6))
        ppool = ctx.enter_context(tc.tile_pool(name="ppool", bufs=8))
        npool = ctx.enter_context(tc.tile_pool(name="npool", bufs=2))
        ypool = ctx.enter_context(tc.tile_pool(name="ypool", bufs=4))
        ps_acc = ctx.enter_context(tc.tile_pool(name="ps_acc", bufs=3, space="PSUM"))
        ps_s = ctx.enter_context(tc.tile_pool(name="ps_s", bufs=3, space="PSUM"))
        ps_o = ctx.enter_context(tc.tile_pool(name="ps_o", bufs=2, space="PSUM"))

        # long-lived tensors
        wqk_sb = glob.tile([128, CB, 2 * HC], F32R)
        wv_sb = glob.tile([128, CB, HC], F32R)
        wo_sb = glob.tile([128, 2, C], F32R)
        xT = glob.tile([128, CB, T], F32R)
        qkT = glob.tile([128, 4, T], BF16)     # [q0 q1 k0 k1] channel blocks
        # (bf16: scores run as pure-bf16 matmuls at the same 1cy/row; the
        # ~2^-9 rounding of Q/K adds ~0.5% attn-weight noise, well within
        # the 2e-2 gate, and halves the qkT footprint)
        v_sb = glob.tile([128, TB, 4, HEAD_DIM + 1], BF16)
        ao_sb = glob.tile([128, 2, T], F32R)   # attn_out^T, 4 heads packed
        ident = glob.tile([128, 128], BF16)
        make_identity(nc, ident)
        vones_f32 = glob.tile([128, TB, 4], F32)
        nc.vector.memset(vones_f32, 1.0)
        nc.vector.tensor_copy(v_sb[:, :, :, HEAD_DIM:], vones_f32[:, :, :, None])
        ones_sb = glob.tile([65, HEAD_DIM], F32R)
        ones_f32 = glob.tile([128, HEAD_DIM], F32)
        nc.vector.memset(ones_f32, 1.0)
        nc.vector.tensor_copy(ones_sb, ones_f32[0:65, :])

        # DMA prefetch: x t-blocks head the critical path; wv is needed at
        # the first V projection (~5us), wqk at B(0) (~10us), wo not until
        # E(0) (~60us). HWDGE drains in issue order.
        xs_tiles = {}

        def fetch_x(tb, split=False):
            xs = xstage.tile([128, C], BF16, tag="xs", name=f"xs{tb}")
            if split:
                nc.sync.dma_start(xs[:, 0:512], xb[tb * 128 : (tb + 1) * 128, 0:512])
                nc.sync.dma_start(xs[:, 512:C], xb[tb * 128 : (tb + 1) * 128, 512:C])
            else:
                nc.sync.dma_start(xs, xb[tb * 128 : (tb + 1) * 128, :])
            xs_tiles[tb] = xs

        wqk_r = wqk.rearrange("(cb p) n -> p cb n", p=128)

        def fetch_wqk(ob):
            nc.sync.dma_start(
                wqk_sb[:, :, ob * 128 : (ob + 1) * 128],
                wqk_r[:, :, ob * 128 : (ob + 1) * 128],
            )

        # The first ~22us is DMA-bus-bound: everything before B(0) totals
        # ~7MB at ~360B/ns. Interleave x t-blocks, wv, and per-ob wqk slices
        # so each PE work item's input lands just before PE reaches it.
        # Heads 0,1 need only wqk slices ob0 (q) and ob2 (k).
        fetch_x(0, split=True)
        fetch_x(1)
        fetch_x(2)
        fetch_x(3)
        # wv in two halves at the same queue position: the V projection's
        # first four accumulation steps start on the first half
        wv_r = wv.rearrange("(cb p) n -> p cb n", p=128)
        nc.sync.dma_start(wv_sb[:, 0:4, :], wv_r[:, 0:4, :])
        nc.sync.dma_start(wv_sb[:, 4:CB, :], wv_r[:, 4:CB, :])
        fetch_wqk(0)
        fetch_wqk(2)
        fetch_x(4)
        fetch_wqk(1)
        fetch_wqk(3)
        fetch_x(5)

        def do_T(tb):
            """Transpose one x t-block into xT (bf16 in, f32r out on evac).

            PSUM cells are 32-bit on TRN2 even for bf16 data, so a bank
            holds 512 elements per partition: 4 transposes per PSUM tile."""
            xs = xs_tiles.pop(tb)
            for half in range(2):
                pt = ps_acc.tile([128, 512], BF16, tag="acc", name=f"pt{tb}_{half}")
                for k in range(4):
                    cb = 4 * half + k
                    nc.tensor.transpose(
                        pt[:, k * 128 : (k + 1) * 128],
                        xs[:, cb * 128 : (cb + 1) * 128],
                        ident,
                    )
                nc.vector.tensor_copy(
                    xT[:, 4 * half : 4 * half + 4, tb * 128 : (tb + 1) * 128],
                    pt.rearrange("p (c t) -> p c t", c=4),
                )
            if 6 <= tb + 5 < TB:
                fetch_x(tb + 5)
            if tb == 4:
                # wo is not needed until E(0) (~45us in); keep it off the
                # critical early x/wqk DMA window
                nc.sync.dma_start(wo_sb, wo.rearrange("(cb p) n -> p cb n", p=128))

        def do_V(tb):
            """Project one t-block's V rows (natural layout)."""
            pv = ps_acc.tile([128, 512], F32, tag="acc", name=f"pv{tb}")
            for cb in range(CB):
                nc.tensor.matmul(
                    pv[:, 0:HC],
                    xT[:, cb, tb * 128 : (tb + 1) * 128],
                    wv_sb[:, cb, :],
                    start=(cb == 0),
                    stop=(cb == CB - 1),
                )
            nc.vector.tensor_copy(
                v_sb[:, tb, :, 0:HEAD_DIM],
                pv[:, 0:HC].rearrange("p (h d) -> p h d", h=4),
            )

        def do_tb(tb):
            do_T(tb)
            do_V(tb)

        def do_B_ob(qc, ob):
            """One 128-channel block of the Qt/Kt projection for chunk qc."""
            pq = ps_acc.tile([128, 512], F32, tag="acc", name=f"pq{qc}_{ob}")
            for cb in range(CB):
                nc.tensor.matmul(
                    pq,
                    wqk_sb[:, cb, ob * 128 : (ob + 1) * 128],
                    xT[:, cb, qc * 512 : (qc + 1) * 512],
                    start=(cb == 0),
                    stop=(cb == CB - 1),
                )
            nc.vector.tensor_copy(qkT[:, ob, qc * 512 : (qc + 1) * 512], pq)

        def tail(h, qc, po):
            # normalize: rows 0..63 attn, row 64 softmax denominators
            hp = (h % 2) * 64
            rf = npool.tile([65, 512], F32R, tag="rf", bufs=1)
            with nc.allow_low_precision(
                reason="softmax denominators round to fp32r for the "
                "normalize broadcast; ~1e-4 relative, within tolerance"
            ):
                nc.vector.reciprocal(rf[64:65, :], po[64:65, :])
            # broadcast the reciprocal row across partitions with a K=1
            # PE matmul against a ones column (engines cannot read with
            # partition stride 0; gpsimd partition_broadcast fails codegen)
            pb = ps_acc.tile([128, 512], F32, tag="acc", name=f"pb{h}_{qc}")
            nc.tensor.matmul(
                pb[0:64, :], ones_sb[64:65, :], rf[64:65, :], start=True, stop=True
            )
            bc = npool.tile([64, 512], F32R, tag="bc", bufs=1)
            nc.vector.tensor_copy(bc, pb[0:64, :])
            if hp == 0:
                nc.vector.tensor_mul(
                    ao_sb[0:64, h // 2, qc * 512 : (qc + 1) * 512],
                    po[0:64, :],
                    bc,
                )
            else:
                aos = npool.tile([64, 512], F32R, tag="aos", bufs=1)
                nc.vector.tensor_mul(aos, po[0:64, :], bc)
                # engines cannot shift partitions; DMA moves 0..63->64..127
                nc.sync.dma_start(
                    ao_sb[64:128, h // 2, qc * 512 : (qc + 1) * 512], aos
                )

        pending = None  # deferred normalize: issued after the NEXT job's
        # matmuls so the PE queue never stalls on the reciprocal chain
        pending_avs = []  # the last AHEAD AV matmuls of a job are issued at
        # the START of the next job, so the inter-job filler work (T/V/B/E)
        # runs during the final exp->AV latency instead of PE stalling

        AHEAD = 4  # scores run this many blocks ahead of the AV consumers so
        # the in-order PE queue never ping-pongs with the Act exp latency

        def flush_avs():
            for fn in pending_avs:
                fn()
            pending_avs.clear()

        def do_job(h, qc):
            nonlocal pending
            flush_avs()
            hp = (h % 2) * 64
            qt = qkT[hp : hp + 64, h // 2, :]
            kt = qkT[hp : hp + 64, 2 + h // 2, :]
            po = ps_o.tile([65, 512], F32, tag="po", name=f"po{h}_{qc}")
            nblocks = 4 * (qc + 1)
            avq = []  # (i, off) AV matmuls not yet issued

            def issue_av(i, off):
                p, pc = ppats[i]
                nc.tensor.matmul(
                    po[:, off:512],
                    v_sb[:, i, h, :],
                    p[:, pc : pc + 512 - off],
                    start=(i == 0),
                    stop=(i == nblocks - 1),
                )

            def diag_select(p, pc):
                # zero above-diagonal within the leading 128-wide square of
                # the block slice starting at column pc
                nc.gpsimd.affine_select(
                    out=p[:, pc : pc + 128],
                    in_=p[:, pc : pc + 128],
                    compare_op=mybir.AluOpType.is_ge,
                    fill=0.0,
                    base=0,
                    pattern=[[1, 128]],
                    channel_multiplier=-1,
                )

            ppats = {}
            for i in range(nblocks - 1):
                r = i - 4 * qc  # >=0 on diagonal blocks
                # v/p are bf16, so the AV matmul runs 1cy/row at any moving
                # width (no fp32r N<256 cliff): diagonal blocks shrink to
                # their true causal width
                off = 0 if r < 0 else 128 * r
                w = 512 - off
                last_pair = i == nblocks - 2  # (r2, r3) share one bank + exp
                ps = ps_s.tile([128, 512], F32, tag="ps", name=f"ps{h}_{qc}_{i}")
                nc.tensor.matmul(
                    ps[:, 0:w],
                    kt[:, i * 128 : (i + 1) * 128],
                    qt[:, qc * 512 + off : (qc + 1) * 512],
                    start=True,
                    stop=True,
                )
                p = ppool.tile([128, 512], BF16, tag="p", name=f"p{h}_{qc}_{i}")
                ppats[i] = (p, 0)
                if last_pair:
                    # r3 scores (width 128) pack right after r2's in the
                    # same PSUM bank; one exp covers both
                    nc.tensor.matmul(
                        ps[:, 256:384],
                        kt[:, (i + 1) * 128 : (i + 2) * 128],
                        qt[:, qc * 512 + 384 : (qc + 1) * 512],
                        start=True,
                        stop=True,
                    )
                    ppats[i + 1] = (p, 256)
                    nc.scalar.activation(
                        p[:, 0:384],
                        ps[:, 0:384],
                        mybir.ActivationFunctionType.Exp,
                        scale=scale,
                    )
                    diag_select(p, 0)
                    diag_select(p, 256)
                    avq.append((i, off))
                    avq.append((i + 1, 384))
                else:
                    nc.scalar.activation(
                        p[:, 0:w],
                        ps[:, 0:w],
                        mybir.ActivationFunctionType.Exp,
                        scale=scale,
                    )
                    if r >= 0:
                        diag_select(p, 0)
                    avq.append((i, off))
                if i >= AHEAD:
                    issue_av(*avq.pop(0))
            # the last AHEAD AVs wait on the exp chain; defer them past the
            # inter-job filler work (flushed at the next job's start)
            for a in avq:
                pending_avs.append(lambda a=a: issue_av(*a))
            if pending is not None:
                tail(*pending)
            pending = (h, qc, po)

        def do_E_ob(qc, ob, pool=None, tag="ps", evac=None):
            """One 128-row block of the out-projection for chunk qc."""
            pool = pool or ps_s
            py = pool.tile([128, 512], F32, tag=tag, name=f"py{qc}_{ob}")
            for cb in range(2):
                nc.tensor.matmul(
                    py,
                    wo_sb[:, cb, ob * 128 : (ob + 1) * 128],
                    ao_sb[:, cb, qc * 512 : (qc + 1) * 512],
                    start=(cb == 0),
                    stop=(cb == 1),
                )
            ys = ypool.tile([128, 512], BF16, tag="ys", name=f"ys{qc}_{ob}")
            (evac or nc.vector.tensor_copy)(ys, py)
            nc.sync.dma_start(
                yt[ob * 128 : (ob + 1) * 128, qc * 512 : (qc + 1) * 512], ys
            )

        # ---- the schedule: one interleaved PE stream, no phase barriers ----
        # Front section paced by DMA arrivals: x t-block transposes and V
        # projections as x lands, B(0) ob-slices as their wqk slices land,
        # and D(0) heads 0/1 as soon as ob0+ob2 are projected.
        do_T(0)
        do_T(1)
        do_T(2)
        do_T(3)
        do_V(0)
        do_V(1)
        do_V(2)
        do_V(3)
        do_B_ob(0, 0)
        do_B_ob(0, 2)
        do_T(4)
        do_B_ob(0, 1)
        do_B_ob(0, 3)
        do_V(4)
        do_job(0, 0)
        do_T(5)
        do_job(1, 0)
        do_V(5)
        do_T(6)
        do_job(2, 0)
        do_V(6)
        do_T(7)
        do_job(3, 0)
        do_V(7)
        do_B_ob(1, 0)
        do_B_ob(1, 2)
        # D(1) with B(1) tail, G2 and E(0) fillers.  T(tb) and V(tb) are
        # always separated by other PE work: V waits on the xT evacuation
        # copy (~1.3us after the transposes), so back-to-back T+V stalls.
        do_job(0, 1)
        do_B_ob(1, 1)
        do_B_ob(1, 3)
        do_job(1, 1)
        do_T(8)
        do_E_ob(0, 0)
        do_E_ob(0, 1)
        do_job(2, 1)
        do_T(9)
        do_V(8)
        do_E_ob(0, 2)
        do_job(3, 1)
        do_T(10)
        do_V(9)
        do_E_ob(0, 3)
        do_E_ob(0, 4)
        # D(2) with G3, B(2) and E(0)/E(1) fillers.  Ordering constraints:
        # job(h,2) needs V(0..11) and B(2, qt/kt obs for its head pair.
        do_T(11)
        do_V(10)
        do_B_ob(2, 0)
        do_B_ob(2, 2)
        do_V(11)
        do_job(0, 2)
        do_B_ob(2, 1)
        do_B_ob(2, 3)
        do_E_ob(0, 5)
        do_job(1, 2)
        do_T(12)
        do_E_ob(0, 6)
        do_E_ob(0, 7)
        do_job(2, 2)
        do_T(13)
        do_V(12)
        do_E_ob(1, 0)
        do_job(3, 2)
        do_T(14)
        do_V(13)
        do_E_ob(1, 1)
        do_T(15)
        do_E_ob(1, 2)
        do_V(14)
        do_E_ob(1, 3)
        do_V(15)
        do_E_ob(1, 4)
        do_E_ob(1, 5)
        # D(3): odd heads first — the final job's tail must not need the
        # ao partition-shift DMA (it would sit on the critical path into
        # E(3)).  B(3) ob1/ob3 (only needed by heads 2,3) and the E(1) tail
        # chunks are pushed into D(3) as fillers: D(3) jobs have the largest
        # Act-vs-PE deficit (the per-exp access overhead scales with nblocks).
        # E evacs inside D(3) stay off the Activation engine (exp-saturated).
        do_B_ob(3, 0)
        do_B_ob(3, 2)
        do_job(1, 3)
        do_B_ob(3, 1)
        do_B_ob(3, 3)
        do_E_ob(1, 6)
        do_job(3, 3)
        do_E_ob(1, 7)
        do_E_ob(2, 0, pool=ps_acc, tag="acc")
        do_E_ob(2, 1)
        do_E_ob(2, 2, pool=ps_acc, tag="acc")
        do_job(0, 3)
        do_E_ob(2, 3)
        do_E_ob(2, 4, pool=ps_acc, tag="acc")
        do_E_ob(2, 5)
        do_job(2, 3)
        do_E_ob(2, 6, pool=ps_acc, tag="acc")
        do_E_ob(2, 7)
        # endgame: the final job's normalize is split into column halves so
        # the first E(3) wave starts while the second half normalizes; E(3)
        # chunks rotate across both PSUM rings and both evac engines
        flush_avs()
        fh, fqc, fpo = pending
        pending = None
        rf = npool.tile([65, 512], F32R, tag="rf", bufs=1, name="rf_fin")
        ysf = [
            ypool.tile([128, 512], BF16, tag="ysf", bufs=8, name=f"ysf{ob}")
            for ob in range(CB)
        ]
        for wave, (c0, c1) in enumerate(((0, 256), (256, 512))):
            with nc.allow_low_precision(reason="fp32r softmax denominators"):
                nc.vector.reciprocal(rf[64:65, c0:c1], fpo[64:65, c0:c1])
            pbf = ps_s.tile([128, 512], F32, tag="ps", name=f"pbf{wave}")
            nc.tensor.matmul(
                pbf[0:64, 0 : c1 - c0],
                ones_sb[64:65, :],
                rf[64:65, c0:c1],
                start=True,
                stop=True,
            )
            bcf = npool.tile([64, 512], F32R, tag="bc", bufs=1, name=f"bcf{wave}")
            nc.vector.tensor_copy(bcf[:, 0 : c1 - c0], pbf[0:64, 0 : c1 - c0])
            nc.vector.tensor_mul(
                ao_sb[0:64, fh // 2, fqc * 512 + c0 : fqc * 512 + c1],
                fpo[0:64, c0:c1],
                bcf[:, 0 : c1 - c0],
            )
            for ob in range(CB):
                pool, tg = (ps_acc, "acc") if ob % 2 == 0 else (ps_s, "ps")
                py = pool.tile([128, 512], F32, tag=tg, name=f"pyf{wave}_{ob}")
                for cb in range(2):
                    nc.tensor.matmul(
                        py[:, 0:256],
                        wo_sb[:, cb, ob * 128 : (ob + 1) * 128],
                        ao_sb[:, cb, fqc * 512 + c0 : fqc * 512 + c1],
                        start=(cb == 0),
                        stop=(cb == 1),
                    )
                (nc.scalar.copy if ob % 2 == 0 else nc.vector.tensor_copy)(
                    ysf[ob][:, c0:c1], py[:, 0:256]
                )
                if wave == 1:
                    # one DMA per ob; the first three take Pool's software-DGE
                    # path (1038ns prep each, serialized on the idle Pool
                    # engine) while the rest drain through HWDGE (625ns each),
                    # so the trailing per-DMA fixed overheads run on two
                    # devices in parallel
                    eng = nc.gpsimd if ob < 3 else nc.sync
                    eng.dma_start(
                        yt[ob * 128 : (ob + 1) * 128,
                           fqc * 512 : (fqc + 1) * 512],
                        ysf[ob],
                    )

    split_multi_waits(nc)
    return nc


_NC_CACHE = None


def kernel(x, W_qkv, W_out):
    global _NC_CACHE
    import ml_dtypes

    x = np.asarray(x, dtype=np.float32).astype(ml_dtypes.bfloat16)
    W_qkv = np.asarray(W_qkv, dtype=np.float32)
    W_out = np.asarray(W_out, dtype=np.float32)

    if _NC_CACHE is None:
        _NC_CACHE = build()
    nc = _NC_CACHE

    in_maps = []
    for core in range(N_CORES):
        b, hg = core // 4, core % 4
        cs = hg * HC
        wq = W_qkv[:, cs : cs + HC]
        wk = W_qkv[:, C + cs : C + cs + HC]
        in_maps.append(
            dict(
                xb=np.ascontiguousarray(x[b]),
                wqk=np.ascontiguousarray(np.concatenate([wq, wk], axis=1)),
                wv=np.ascontiguousarray(W_qkv[:, 2 * C + cs : 2 * C + cs + HC]),
                wo=np.ascontiguousarray(W_out[cs : cs + HC, :]),
            )
        )

    res = run_bass_kernel_spmd(nc, in_maps, core_ids=list(range(N_CORES)))
    out = np.zeros((B, T, C), dtype=np.float32)
    for core in range(N_CORES):
        out[core // 4] += res.results[core]["yt"].astype(np.float32).T
    return out


# revision 84
# speedup vs baseline: 1.0144x; 1.0144x over previous
"""Causal self-attention Trainium2 kernel (8 NeuronCores).

Reference computation (fp32):
    qkv = x @ W_qkv; q,k,v = split(qkv)
    per head: scores = q k^T / sqrt(64), causal softmax, out = attn @ v
    y = out @ W_out

Sharding: 8 cores = 2 batches x 4 head-groups. Core c handles batch
b = c // 4 and heads [4*hg, 4*hg+4) with hg = c % 4. Each core computes
a partial y^T (its 4 heads' contribution through W_out rows); the host
sums the 4 partials per batch.

Fully software-pipelined single schedule (185149 -> 156566 ns in the
TimelineSim cost model): transposes/V-proj/QK-proj groups, attention
jobs and out-projection chunks are interleaved in one PE instruction
stream so the PE never drains between phases; everything is paced by
just-in-time DMA arrival at the front and drains through two DMA issue
paths (HWDGE + Pool soft-DGE) at the tail.

Dataflow per core (projection matmuls fp32r ~= TF32; x, Q^T/K^T, V and
attention weights bf16; PSUM accumulation fp32):
  A. x (bf16, host-cast) -> PE-transpose -> xT [c, t] upconverted to
     f32r on evacuation; 4 transposes per PSUM bank. (fp32 transposes
     cost 2cy/row and f32r transposes fail neuronxcc codegen.)
  B. Qt/Kt = (W_qk^T x^T) directly in [channel, t] layout, bf16.
  C. V natural [t, channel] bf16; ones column at 64 per head (softmax
     denominator accumulates in the AV matmul's row 64 for free).
  D. per (head, q-chunk of 512): S^T blocks = Kt_blk^T Qt_chunk (K=64),
     P = exp(S/8) with the above-diagonal 128-wide square zeroed by a
     Pool affine_select on P; the two smallest diagonal blocks (r2,r3)
     share one PSUM bank and one exp instruction (the ~185ns per-exp
     SBUF access overhead is what paces the attention phase).
     O_aug = V_aug^T P accumulated over s-blocks. Normalize: DVE
     reciprocal of row 64, broadcast across partitions via a K=1 PE
     matmul against a ones column (engines cannot read partition-
     stride-0; DVE cannot read two PSUM operands), DVE row-mul.
     Odd heads DMA-shift rows to partitions 64..127. The normalize and
     each job's last AHEAD AV matmuls are deferred past the next job's
     filler work so PE never waits on the exp/reciprocal chains.
  E. yT[c_out, t] = W_out_slice^T @ attn_outT (K=128 over 2 blocks),
     spread through D as PE filler; y leaves as bf16 (the host
     upconverts and sums partials in fp32). The final q-chunk's
     normalize is split into column halves so its out-projection waves
     overlap the second half's normalize chain.

Scores are O(1) (x ~ N(0,1), W scaled 1/sqrt(1024)), |s| < ~8, so
softmax max-subtraction is skipped; exp is computed directly. Masked
positions exp to finite garbage and are zeroed by the affine_select.

This container's walrus accepts at most ONE on_wait per instruction while
Tile emits several; split_multi_waits() legalizes the program after
TileContext exit.
"""

import math
from contextlib import ExitStack

import numpy as np

import concourse.bass as bass
import concourse.mybir as mybir
import concourse.tile as tile
from concourse.bass_utils import run_bass_kernel_spmd
from concourse.masks import make_identity

F32 = mybir.dt.float32
F32R = mybir.dt.float32r
BF16 = mybir.dt.bfloat16

B, T, C = 2, 2048, 1024
N_HEADS, HEAD_DIM = 16, 64
HEADS_PER_CORE = 4          # 4 heads/core (16 heads / 4 head-groups)
HC = HEADS_PER_CORE * HEAD_DIM  # 256 channels per core
N_CORES = 8
TB = T // 128               # 16 t-blocks of 128
QC = T // 512               # 4 q-chunks of 512
CB = C // 128               # 8 c_in blocks


def split_multi_waits(nc):
    """Walrus here allows only one on_wait per instruction; move extras to
    standalone EventSemaphore instructions on the same engine."""
    n_split = 0
    for fn in nc.m.functions:
        for bb in fn.blocks:
            if not any(
                inst.sync_info is not None and len(inst.sync_info.on_wait) > 1
                for inst in bb.instructions
            ):
                continue
            out = []
            for inst in bb.instructions:
                si = inst.sync_info
                if si is not None and len(si.on_wait) > 1:
                    waits = list(si.on_wait)
                    for i, w in enumerate(waits[:-1]):
                        out.append(
                            mybir.InstEventSemaphore(
                                name=f"{inst.name}_sw{i}",
                                engine=inst.engine,
                                sync_info=mybir.SyncInfo(on_wait=[w], on_update=[]),
                            )
                        )
                        n_split += 1
                    inst.sync_info = mybir.SyncInfo(
                        on_wait=[waits[-1]], on_update=list(si.on_update)
                    )
                out.append(inst)
            bb.instructions = out
    return n_split


def build():
    nc = bass.Bass(trn_type="TRN2")
    # x arrives as bf16 (host-cast): halves the front-critical x DMA bytes
    # and makes the PE transposes 1.0 cy/row (fp32 is 2.0; f32r transposes
    # fail neuronxcc codegen). xT is upconverted to f32r on evacuation, so
    # all downstream matmuls stay fp32r.
    xb = nc.dram_tensor("xb", [T, C], BF16, kind="ExternalInput")
    wqk = nc.dram_tensor("wqk", [C, 2 * HC], F32R, kind="ExternalInput")
    wv = nc.dram_tensor("wv", [C, HC], F32R, kind="ExternalInput")
    wo = nc.dram_tensor("wo", [HC, C], F32R, kind="ExternalInput")
    # y partials leave the core as bf16 (halves the trailing output-DMA
    # serialization); the host upconverts and sums partials in fp32
    yt = nc.dram_tensor("yt", [C, T], BF16, kind="ExternalOutput")

    scale = 1.0 / math.sqrt(HEAD_DIM)

    with tile.TileContext(nc) as tc, ExitStack() as ctx:
        glob = ctx.enter_context(tc.tile_pool(name="glob", bufs=1))
        xstage = ctx.enter_context(tc.tile_pool(name="xstage", bufs=6))
        ppool = ctx.enter_context(tc.tile_pool(name="ppool", bufs=8))
        npool = ctx.enter_context(tc.tile_pool(name="npool", bufs=2))
        ypool = ctx.enter_context(tc.tile_pool(name="ypool", bufs=4))
        ps_acc = ctx.enter_context(tc.tile_pool(name="ps_acc", bufs=2, space="PSUM"))
        ps_s = ctx.enter_context(tc.tile_pool(name="ps_s", bufs=4, space="PSUM"))
        ps_o = ctx.enter_context(tc.tile_pool(name="ps_o", bufs=2, space="PSUM"))

        # long-lived tensors
        wqk_sb = glob.tile([128, CB, 2 * HC], F32R)
        wv_sb = glob.tile([128, CB, HC], F32R)
        wo_sb = glob.tile([128, 2, C], F32R)
        xT = glob.tile([128, CB, T], F32R)
        qkT = glob.tile([128, 4, T], BF16)     # [q0 q1 k0 k1] channel blocks
        # (bf16: scores run as pure-bf16 matmuls at the same 1cy/row; the
        # ~2^-9 rounding of Q/K adds ~0.5% attn-weight noise, well within
        # the 2e-2 gate, and halves the qkT footprint)
        v_sb = glob.tile([128, TB, 4, HEAD_DIM + 1], BF16)
        ao_sb = glob.tile([128, 2, T], F32R)   # attn_out^T, 4 heads packed
        ident = glob.tile([128, 128], BF16)
        make_identity(nc, ident)
        vones_f32 = glob.tile([128, TB, 4], F32)
        nc.vector.memset(vones_f32, 1.0)
        nc.vector.tensor_copy(v_sb[:, :, :, HEAD_DIM:], vones_f32[:, :, :, None])
        ones_sb = glob.tile([65, HEAD_DIM], F32R)
        ones_f32 = glob.tile([128, HEAD_DIM], F32)
        nc.vector.memset(ones_f32, 1.0)
        nc.vector.tensor_copy(ones_sb, ones_f32[0:65, :])

        # DMA prefetch: x t-blocks head the critical path; wv is needed at
        # the first V projection (~5us), wqk at B(0) (~10us), wo not until
        # E(0) (~60us). HWDGE drains in issue order.
        xs_tiles = {}

        def fetch_x(tb, split=False):
            xs = xstage.tile([128, C], BF16, tag="xs", name=f"xs{tb}")
            if split:
                nc.sync.dma_start(xs[:, 0:512], xb[tb * 128 : (tb + 1) * 128, 0:512])
                nc.sync.dma_start(xs[:, 512:C], xb[tb * 128 : (tb + 1) * 128, 512:C])
            else:
                nc.sync.dma_start(xs, xb[tb * 128 : (tb + 1) * 128, :])
            xs_tiles[tb] = xs

        wqk_r = wqk.rearrange("(cb p) n -> p cb n", p=128)

        def fetch_wqk(ob):
            nc.sync.dma_start(
                wqk_sb[:, :, ob * 128 : (ob + 1) * 128],
                wqk_r[:, :, ob * 128 : (ob + 1) * 128],
            )

        # The first ~22us is DMA-bus-bound: everything before B(0) totals
        # ~7MB at ~360B/ns. Interleave x t-blocks, wv, and per-ob wqk slices
        # so each PE work item's input lands just before PE reaches it.
        # Heads 0,1 need only wqk slices ob0 (q) and ob2 (k).
        fetch_x(0, split=True)
        fetch_x(1)
        fetch_x(2)
        fetch_x(3)
        # wv in two halves at the same queue position: the V projection's
        # first four accumulation steps start on the first half
        wv_r = wv.rearrange("(cb p) n -> p cb n", p=128)
        nc.sync.dma_start(wv_sb[:, 0:4, :], wv_r[:, 0:4, :])
        nc.sync.dma_start(wv_sb[:, 4:CB, :], wv_r[:, 4:CB, :])
        fetch_wqk(0)
        fetch_wqk(2)
        fetch_x(4)
        fetch_wqk(1)
        fetch_wqk(3)
        fetch_x(5)

        def do_T(tb):
            """Transpose one x t-block into xT (bf16 in, f32r out on evac).

            PSUM cells are 32-bit on TRN2 even for bf16 data, so a bank
            holds 512 elements per partition: 4 transposes per PSUM tile."""
            xs = xs_tiles.pop(tb)
            for half in range(2):
                pt = ps_acc.tile([128, 512], BF16, tag="acc", name=f"pt{tb}_{half}")
                for k in range(4):
                    cb = 4 * half + k
                    nc.tensor.transpose(
                        pt[:, k * 128 : (k + 1) * 128],
                        xs[:, cb * 128 : (cb + 1) * 128],
                        ident,
                    )
                nc.vector.tensor_copy(
                    xT[:, 4 * half : 4 * half + 4, tb * 128 : (tb + 1) * 128],
                    pt.rearrange("p (c t) -> p c t", c=4),
                )
            if 6 <= tb + 5 < TB:
                fetch_x(tb + 5)
            if tb == 4:
                # wo is not needed until E(0) (~45us in); keep it off the
                # critical early x/wqk DMA window
                nc.sync.dma_start(wo_sb, wo.rearrange("(cb p) n -> p cb n", p=128))

        def do_V(tb):
            """Project one t-block's V rows (natural layout)."""
            pv = ps_acc.tile([128, 512], F32, tag="acc", name=f"pv{tb}")
            for cb in range(CB):
                nc.tensor.matmul(
                    pv[:, 0:HC],
                    xT[:, cb, tb * 128 : (tb + 1) * 128],
                    wv_sb[:, cb, :],
                    start=(cb == 0),
                    stop=(cb == CB - 1),
                )
            nc.vector.tensor_copy(
                v_sb[:, tb, :, 0:HEAD_DIM],
                pv[:, 0:HC].rearrange("p (h d) -> p h d", h=4),
            )

        def do_tb(tb):
            do_T(tb)
            do_V(tb)

        def do_B_ob(qc, ob):
            """One 128-channel block of the Qt/Kt projection for chunk qc."""
            pq = ps_acc.tile([128, 512], F32, tag="acc", name=f"pq{qc}_{ob}")
            for cb in range(CB):
                nc.tensor.matmul(
                    pq,
                    wqk_sb[:, cb, ob * 128 : (ob + 1) * 128],
                    xT[:, cb, qc * 512 : (qc + 1) * 512],
                    start=(cb == 0),
                    stop=(cb == CB - 1),
                )
            nc.vector.tensor_copy(qkT[:, ob, qc * 512 : (qc + 1) * 512], pq)

        def tail(h, qc, po):
            # normalize: rows 0..63 attn, row 64 softmax denominators
            hp = (h % 2) * 64
            rf = npool.tile([65, 512], F32R, tag="rf", bufs=1)
            with nc.allow_low_precision(
                reason="softmax denominators round to fp32r for the "
                "normalize broadcast; ~1e-4 relative, within tolerance"
            ):
                nc.vector.reciprocal(rf[64:65, :], po[64:65, :])
            # broadcast the reciprocal row across partitions with a K=1
            # PE matmul against a ones column (engines cannot read with
            # partition stride 0; gpsimd partition_broadcast fails codegen)
            pb = ps_acc.tile([128, 512], F32, tag="acc", name=f"pb{h}_{qc}")
            nc.tensor.matmul(
                pb[0:64, :], ones_sb[64:65, :], rf[64:65, :], start=True, stop=True
            )
            bc = npool.tile([64, 512], F32R, tag="bc", bufs=1)
            nc.vector.tensor_copy(bc, pb[0:64, :])
            if hp == 0:
                nc.vector.tensor_mul(
                    ao_sb[0:64, h // 2, qc * 512 : (qc + 1) * 512],
                    po[0:64, :],
                    bc,
                )
            else:
                aos = npool.tile([64, 512], F32R, tag="aos", bufs=1)
                nc.vector.tensor_mul(aos, po[0:64, :], bc)
                # engines cannot shift partitions; DMA moves 0..63->64..127
                nc.sync.dma_start(
                    ao_sb[64:128, h // 2, qc * 512 : (qc + 1) * 512], aos
                )

        pending = None  # deferred normalize: issued after the NEXT job's
        # matmuls so the PE queue never stalls on the reciprocal chain
        pending_avs = []  # the last AHEAD AV matmuls of a job are issued at
        # the START of the next job, so the inter-job filler work (T/V/B/E)
        # runs during the final exp->AV latency instead of PE stalling

        AHEAD = 4  # scores run this many blocks ahead of the AV consumers so
        # the in-order PE queue never ping-pongs with the Act exp latency

        def flush_avs():
            for fn in pending_avs:
                fn()
            pending_avs.clear()

        def do_job(h, qc):
            nonlocal pending
            flush_avs()
            hp = (h % 2) * 64
            qt = qkT[hp : hp + 64, h // 2, :]
            kt = qkT[hp : hp + 64, 2 + h // 2, :]
            po = ps_o.tile([65, 512], F32, tag="po", name=f"po{h}_{qc}")
            nblocks = 4 * (qc + 1)
            avq = []  # (i, off) AV matmuls not yet issued

            def issue_av(i, off):
                p, pc = ppats[i]
                nc.tensor.matmul(
                    po[:, off:512],
                    v_sb[:, i, h, :],
                    p[:, pc : pc + 512 - off],
                    start=(i == 0),
                    stop=(i == nblocks - 1),
                )

            def diag_select(p, pc):
                # zero above-diagonal within the leading 128-wide square of
                # the block slice starting at column pc
                nc.gpsimd.affine_select(
                    out=p[:, pc : pc + 128],
                    in_=p[:, pc : pc + 128],
                    compare_op=mybir.AluOpType.is_ge,
                    fill=0.0,
                    base=0,
                    pattern=[[1, 128]],
                    channel_multiplier=-1,
                )

            ppats = {}
            for i in range(nblocks - 1):
                r = i - 4 * qc  # >=0 on diagonal blocks
                # v/p are bf16, so the AV matmul runs 1cy/row at any moving
                # width (no fp32r N<256 cliff): diagonal blocks shrink to
                # their true causal width
                off = 0 if r < 0 else 128 * r
                w = 512 - off
                last_pair = i == nblocks - 2  # (r2, r3) share one bank + exp
                ps = ps_s.tile([128, 512], F32, tag="ps", name=f"ps{h}_{qc}_{i}")
                nc.tensor.matmul(
                    ps[:, 0:w],
                    kt[:, i * 128 : (i + 1) * 128],
                    qt[:, qc * 512 + off : (qc + 1) * 512],
                    start=True,
                    stop=True,
                )
                p = ppool.tile([128, 512], BF16, tag="p", name=f"p{h}_{qc}_{i}")
                ppats[i] = (p, 0)
                if last_pair:
                    # r3 scores (width 128) pack right after r2's in the
                    # same PSUM bank; one exp covers both
                    nc.tensor.matmul(
                        ps[:, 256:384],
                        kt[:, (i + 1) * 128 : (i + 2) * 128],
                        qt[:, qc * 512 + 384 : (qc + 1) * 512],
                        start=True,
                        stop=True,
                    )
                    ppats[i + 1] = (p, 256)
                    nc.scalar.activation(
                        p[:, 0:384],
                        ps[:, 0:384],
                        mybir.ActivationFunctionType.Exp,
                        scale=scale,
                    )
                    diag_select(p, 0)
                    diag_select(p, 256)
                    avq.append((i, off))
                    avq.append((i + 1, 384))
                else:
                    nc.scalar.activation(
                        p[:, 0:w],
                        ps[:, 0:w],
                        mybir.ActivationFunctionType.Exp,
                        scale=scale,
                    )
                    if r >= 0:
                        diag_select(p, 0)
                    avq.append((i, off))
                if i >= AHEAD:
                    issue_av(*avq.pop(0))
            # the last AHEAD AVs wait on the exp chain; defer them past the
            # inter-job filler work (flushed at the next job's start)
            for a in avq:
                pending_avs.append(lambda a=a: issue_av(*a))
            if pending is not None:
                tail(*pending)
            pending = (h, qc, po)

        def do_E_ob(qc, ob, pool=None, tag="ps", evac=None):
            """One 128-row block of the out-projection for chunk qc."""
            pool = pool or ps_s
            py = pool.tile([128, 512], F32, tag=tag, name=f"py{qc}_{ob}")
            for cb in range(2):
                nc.tensor.matmul(
                    py,
                    wo_sb[:, cb, ob * 128 : (ob + 1) * 128],
                    ao_sb[:, cb, qc * 512 : (qc + 1) * 512],
                    start=(cb == 0),
                    stop=(cb == 1),
                )
            ys = ypool.tile([128, 512], BF16, tag="ys", name=f"ys{qc}_{ob}")
            (evac or nc.vector.tensor_copy)(ys, py)
            nc.sync.dma_start(
                yt[ob * 128 : (ob + 1) * 128, qc * 512 : (qc + 1) * 512], ys
            )

        # ---- the schedule: one interleaved PE stream, no phase barriers ----
        # Front section paced by DMA arrivals: x t-block transposes and V
        # projections as x lands, B(0) ob-slices as their wqk slices land,
        # and D(0) heads 0/1 as soon as ob0+ob2 are projected.
        do_T(0)
        do_T(1)
        do_T(2)
        do_T(3)
        do_V(0)
        do_V(1)
        do_V(2)
        do_V(3)
        do_B_ob(0, 0)
        do_B_ob(0, 2)
        do_T(4)
        do_B_ob(0, 1)
        do_B_ob(0, 3)
        do_V(4)
        do_job(0, 0)
        do_T(5)
        do_job(1, 0)
        do_V(5)
        do_T(6)
        do_job(2, 0)
        do_V(6)
        do_T(7)
        do_job(3, 0)
        do_V(7)
        do_B_ob(1, 0)
        do_B_ob(1, 2)
        # D(1) with B(1) tail, G2 and E(0) fillers.  T(tb) and V(tb) are
        # always separated by other PE work: V waits on the xT evacuation
        # copy (~1.3us after the transposes), so back-to-back T+V stalls.
        do_job(0, 1)
        do_B_ob(1, 1)
        do_B_ob(1, 3)
        do_job(1, 1)
        do_T(8)
        do_E_ob(0, 0)
        do_E_ob(0, 1)
        do_job(2, 1)
        do_T(9)
        do_V(8)
        do_E_ob(0, 2)
        do_job(3, 1)
        do_T(10)
        do_V(9)
        do_E_ob(0, 3)
        do_E_ob(0, 4)
        # D(2) with G3, B(2) and E(0)/E(1) fillers.  Ordering constraints:
        # job(h,2) needs V(0..11) and B(2, qt/kt obs for its head pair.
        do_T(11)
        do_V(10)
        do_B_ob(2, 0)
        do_B_ob(2, 2)
        do_V(11)
        do_job(0, 2)
        do_B_ob(2, 1)
        do_B_ob(2, 3)
        do_E_ob(0, 5)
        do_job(1, 2)
        do_T(12)
        do_E_ob(0, 6)
        do_E_ob(0, 7)
        do_job(2, 2)
        do_T(13)
        do_V(12)
        do_E_ob(1, 0)
        do_job(3, 2)
        do_T(14)
        do_V(13)
        do_E_ob(1, 1)
        do_T(15)
        do_E_ob(1, 2)
        do_V(14)
        do_E_ob(1, 3)
        do_V(15)
        # D(3): odd heads first — the final job's tail must not need the
        # ao partition-shift DMA (it would sit on the critical path into
        # E(3)).  B(3) ob1/ob3 (only needed by heads 2,3) and the E(1) tail
        # chunks are pushed into D(3) as fillers: D(3) jobs have the largest
        # Act-vs-PE deficit (the per-exp access overhead scales with nblocks).
        # E evacs inside D(3) stay off the Activation engine (exp-saturated).
        do_B_ob(3, 0)
        do_B_ob(3, 2)
        do_job(1, 3)
        do_B_ob(3, 1)
        do_B_ob(3, 3)
        do_E_ob(1, 6)
        do_job(3, 3)
        do_E_ob(1, 7)
        do_E_ob(2, 0, pool=ps_acc, tag="acc")
        do_E_ob(2, 1)
        do_E_ob(2, 2, pool=ps_acc, tag="acc")
        do_job(0, 3)
        do_E_ob(1, 4)
        do_E_ob(2, 3)
        do_E_ob(2, 4, pool=ps_acc, tag="acc")
        do_E_ob(2, 5)
        do_job(2, 3)
        do_E_ob(1, 5)
        do_E_ob(2, 6, pool=ps_acc, tag="acc")
        do_E_ob(2, 7)
        # endgame: the final job's normalize is split into column halves so
        # the first E(3) wave starts while the second half normalizes; E(3)
        # chunks rotate across both PSUM rings and both evac engines
        flush_avs()
        fh, fqc, fpo = pending
        pending = None
        rf = npool.tile([65, 512], F32R, tag="rf", bufs=1, name="rf_fin")
        ysf = [
            ypool.tile([128, 512], BF16, tag="ysf", bufs=8, name=f"ysf{ob}")
            for ob in range(CB)
        ]
        for wave, (c0, c1) in enumerate(((0, 256), (256, 512))):
            with nc.allow_low_precision(reason="fp32r softmax denominators"):
                nc.vector.reciprocal(rf[64:65, c0:c1], fpo[64:65, c0:c1])
            pbf = ps_s.tile([128, 512], F32, tag="ps", name=f"pbf{wave}")
            nc.tensor.matmul(
                pbf[0:64, 0 : c1 - c0],
                ones_sb[64:65, :],
                rf[64:65, c0:c1],
                start=True,
                stop=True,
            )
            bcf = npool.tile([64, 512], F32R, tag="bc", bufs=1, name=f"bcf{wave}")
            nc.vector.tensor_copy(bcf[:, 0 : c1 - c0], pbf[0:64, 0 : c1 - c0])
            nc.vector.tensor_mul(
                ao_sb[0:64, fh // 2, fqc * 512 + c0 : fqc * 512 + c1],
                fpo[0:64, c0:c1],
                bcf[:, 0 : c1 - c0],
            )
            for ob in range(CB):
                pool, tg = (ps_acc, "acc") if ob % 2 == 0 else (ps_s, "ps")
                py = pool.tile([128, 512], F32, tag=tg, name=f"pyf{wave}_{ob}")
                for cb in range(2):
                    nc.tensor.matmul(
                        py[:, 0:256],
                        wo_sb[:, cb, ob * 128 : (ob + 1) * 128],
                        ao_sb[:, cb, fqc * 512 + c0 : fqc * 512 + c1],
                        start=(cb == 0),
                        stop=(cb == 1),
                    )
                (nc.scalar.copy if ob % 2 == 0 else nc.vector.tensor_copy)(
                    ysf[ob][:, c0:c1], py[:, 0:256]
                )
                if wave == 1:
                    # one DMA per ob; the first three take Pool's software-DGE
                    # path (1038ns prep each, serialized on the idle Pool
                    # engine) while the rest drain through HWDGE (625ns each),
                    # so the trailing per-DMA fixed overheads run on two
                    # devices in parallel
                    eng = nc.gpsimd if ob < 3 else nc.sync
                    eng.dma_start(
                        yt[ob * 128 : (ob + 1) * 128,
                           fqc * 512 : (fqc + 1) * 512],
                        ysf[ob],
                    )

    split_multi_waits(nc)
    return nc


_NC_CACHE = None


def kernel(x, W_qkv, W_out):
    global _NC_CACHE
    import ml_dtypes

    x = np.asarray(x, dtype=np.float32).astype(ml_dtypes.bfloat16)
    W_qkv = np.asarray(W_qkv, dtype=np.float32)
    W_out = np.asarray(W_out, dtype=np.float32)

    if _NC_CACHE is None:
        _NC_CACHE = build()
    nc = _NC_CACHE

    in_maps = []
    for core in range(N_CORES):
        b, hg = core // 4, core % 4
        cs = hg * HC
        wq = W_qkv[:, cs : cs + HC]
        wk = W_qkv[:, C + cs : C + cs + HC]
        in_maps.append(
            dict(
                xb=np.ascontiguousarray(x[b]),
                wqk=np.ascontiguousarray(np.concatenate([wq, wk], axis=1)),
                wv=np.ascontiguousarray(W_qkv[:, 2 * C + cs : 2 * C + cs + HC]),
                wo=np.ascontiguousarray(W_out[cs : cs + HC, :]),
            )
        )

    res = run_bass_kernel_spmd(nc, in_maps, core_ids=list(range(N_CORES)))
    out = np.zeros((B, T, C), dtype=np.float32)
    for core in range(N_CORES):
        out[core // 4] += res.results[core]["yt"].astype(np.float32).T
    return out


# revision 85
# speedup vs baseline: 1.0346x; 1.0199x over previous
"""Causal self-attention Trainium2 kernel (8 NeuronCores).

Reference computation (fp32):
    qkv = x @ W_qkv; q,k,v = split(qkv)
    per head: scores = q k^T / sqrt(64), causal softmax, out = attn @ v
    y = out @ W_out

Sharding: 8 cores = 2 batches x 4 head-groups. Core c handles batch
b = c // 4 and heads [4*hg, 4*hg+4) with hg = c % 4. Each core computes
a partial y^T (its 4 heads' contribution through W_out rows); the host
sums the 4 partials per batch.

Fully software-pipelined single schedule (185149 -> 156566 ns in the
TimelineSim cost model): transposes/V-proj/QK-proj groups, attention
jobs and out-projection chunks are interleaved in one PE instruction
stream so the PE never drains between phases; everything is paced by
just-in-time DMA arrival at the front and drains through two DMA issue
paths (HWDGE + Pool soft-DGE) at the tail.

Dataflow per core (projection matmuls fp32r ~= TF32; x, Q^T/K^T, V and
attention weights bf16; PSUM accumulation fp32):
  A. x (bf16, host-cast) -> PE-transpose -> xT [c, t] upconverted to
     f32r on evacuation; 4 transposes per PSUM bank. (fp32 transposes
     cost 2cy/row and f32r transposes fail neuronxcc codegen.)
  B. Qt/Kt = (W_qk^T x^T) directly in [channel, t] layout, bf16.
  C. V natural [t, channel] bf16; ones column at 64 per head (softmax
     denominator accumulates in the AV matmul's row 64 for free).
  D. per (head, q-chunk of 512): S^T blocks = Kt_blk^T Qt_chunk (K=64),
     P = exp(S/8) with the above-diagonal 128-wide square zeroed by a
     Pool affine_select on P; the two smallest diagonal blocks (r2,r3)
     share one PSUM bank and one exp instruction (the ~185ns per-exp
     SBUF access overhead is what paces the attention phase).
     O_aug = V_aug^T P accumulated over s-blocks. Normalize: DVE
     reciprocal of row 64, broadcast across partitions via a K=1 PE
     matmul against a ones column (engines cannot read partition-
     stride-0; DVE cannot read two PSUM operands), DVE row-mul.
     Odd heads DMA-shift rows to partitions 64..127. The normalize and
     each job's last AHEAD AV matmuls are deferred past the next job's
     filler work so PE never waits on the exp/reciprocal chains.
  E. yT[c_out, t] = W_out_slice^T @ attn_outT (K=128 over 2 blocks),
     spread through D as PE filler; y leaves as bf16 (the host
     upconverts and sums partials in fp32). The final q-chunk's
     normalize is split into column halves so its out-projection waves
     overlap the second half's normalize chain.

Scores are O(1) (x ~ N(0,1), W scaled 1/sqrt(1024)), |s| < ~8, so
softmax max-subtraction is skipped; exp is computed directly. Masked
positions exp to finite garbage and are zeroed by the affine_select.

This container's walrus accepts at most ONE on_wait per instruction while
Tile emits several; split_multi_waits() legalizes the program after
TileContext exit.
"""

import math
from contextlib import ExitStack

import numpy as np

import concourse.bass as bass
import concourse.mybir as mybir
import concourse.tile as tile
from concourse.bass_utils import run_bass_kernel_spmd
from concourse.masks import make_identity

F32 = mybir.dt.float32
F32R = mybir.dt.float32r
BF16 = mybir.dt.bfloat16

B, T, C = 2, 2048, 1024
N_HEADS, HEAD_DIM = 16, 64
HEADS_PER_CORE = 4          # 4 heads/core (16 heads / 4 head-groups)
HC = HEADS_PER_CORE * HEAD_DIM  # 256 channels per core
N_CORES = 8
TB = T // 128               # 16 t-blocks of 128
QC = T // 512               # 4 q-chunks of 512
CB = C // 128               # 8 c_in blocks


def split_multi_waits(nc):
    """Walrus here allows only one on_wait per instruction; move extras to
    standalone EventSemaphore instructions on the same engine."""
    n_split = 0
    for fn in nc.m.functions:
        for bb in fn.blocks:
            if not any(
                inst.sync_info is not None and len(inst.sync_info.on_wait) > 1
                for inst in bb.instructions
            ):
                continue
            out = []
            for inst in bb.instructions:
                si = inst.sync_info
                if si is not None and len(si.on_wait) > 1:
                    waits = list(si.on_wait)
                    for i, w in enumerate(waits[:-1]):
                        out.append(
                            mybir.InstEventSemaphore(
                                name=f"{inst.name}_sw{i}",
                                engine=inst.engine,
                                sync_info=mybir.SyncInfo(on_wait=[w], on_update=[]),
                            )
                        )
                        n_split += 1
                    inst.sync_info = mybir.SyncInfo(
                        on_wait=[waits[-1]], on_update=list(si.on_update)
                    )
                out.append(inst)
            bb.instructions = out
    return n_split


def build():
    nc = bass.Bass(trn_type="TRN2")
    # x arrives as bf16 (host-cast): halves the front-critical x DMA bytes
    # and makes the PE transposes 1.0 cy/row (fp32 is 2.0; f32r transposes
    # fail neuronxcc codegen). xT is upconverted to f32r on evacuation, so
    # all downstream matmuls stay fp32r.
    xb = nc.dram_tensor("xb", [T, C], BF16, kind="ExternalInput")
    wqk = nc.dram_tensor("wqk", [C, 2 * HC], BF16, kind="ExternalInput")
    wv = nc.dram_tensor("wv", [C, HC], BF16, kind="ExternalInput")
    wo = nc.dram_tensor("wo", [HC, C], F32R, kind="ExternalInput")
    # y partials leave the core as bf16 (halves the trailing output-DMA
    # serialization); the host upconverts and sums partials in fp32
    yt = nc.dram_tensor("yt", [C, T], BF16, kind="ExternalOutput")

    scale = 1.0 / math.sqrt(HEAD_DIM)

    with tile.TileContext(nc) as tc, ExitStack() as ctx:
        glob = ctx.enter_context(tc.tile_pool(name="glob", bufs=1))
        xstage = ctx.enter_context(tc.tile_pool(name="xstage", bufs=6))
        ppool = ctx.enter_context(tc.tile_pool(name="ppool", bufs=8))
        npool = ctx.enter_context(tc.tile_pool(name="npool", bufs=2))
        ypool = ctx.enter_context(tc.tile_pool(name="ypool", bufs=4))
        ps_acc = ctx.enter_context(tc.tile_pool(name="ps_acc", bufs=2, space="PSUM"))
        ps_s = ctx.enter_context(tc.tile_pool(name="ps_s", bufs=4, space="PSUM"))
        ps_o = ctx.enter_context(tc.tile_pool(name="ps_o", bufs=2, space="PSUM"))

        # long-lived tensors
        wqk_sb = glob.tile([128, CB, 2 * HC], BF16)
        wv_sb = glob.tile([128, CB, HC], BF16)
        wo_sb = glob.tile([128, 2, C], F32R)
        xT = glob.tile([128, CB, T], BF16)
        qkT = glob.tile([128, 4, T], BF16)     # [q0 q1 k0 k1] channel blocks
        # (bf16: scores run as pure-bf16 matmuls at the same 1cy/row; the
        # ~2^-9 rounding of Q/K adds ~0.5% attn-weight noise, well within
        # the 2e-2 gate, and halves the qkT footprint)
        v_sb = glob.tile([128, TB, 4, HEAD_DIM + 1], BF16)
        ao_sb = glob.tile([128, 2, T], F32R)   # attn_out^T, 4 heads packed
        ident = glob.tile([128, 128], BF16)
        make_identity(nc, ident)
        vones_f32 = glob.tile([128, TB, 4], F32)
        nc.vector.memset(vones_f32, 1.0)
        nc.vector.tensor_copy(v_sb[:, :, :, HEAD_DIM:], vones_f32[:, :, :, None])
        ones_sb = glob.tile([65, HEAD_DIM], F32R)
        ones_f32 = glob.tile([128, HEAD_DIM], F32)
        nc.vector.memset(ones_f32, 1.0)
        nc.vector.tensor_copy(ones_sb, ones_f32[0:65, :])

        # DMA prefetch: x t-blocks head the critical path; wv is needed at
        # the first V projection (~5us), wqk at B(0) (~10us), wo not until
        # E(0) (~60us). HWDGE drains in issue order.
        xs_tiles = {}

        def fetch_x(tb, split=False):
            xs = xstage.tile([128, C], BF16, tag="xs", name=f"xs{tb}")
            if split:
                nc.sync.dma_start(xs[:, 0:512], xb[tb * 128 : (tb + 1) * 128, 0:512])
                nc.sync.dma_start(xs[:, 512:C], xb[tb * 128 : (tb + 1) * 128, 512:C])
            else:
                nc.sync.dma_start(xs, xb[tb * 128 : (tb + 1) * 128, :])
            xs_tiles[tb] = xs

        wqk_r = wqk.rearrange("(cb p) n -> p cb n", p=128)

        def fetch_wqk(ob):
            nc.sync.dma_start(
                wqk_sb[:, :, ob * 128 : (ob + 1) * 128],
                wqk_r[:, :, ob * 128 : (ob + 1) * 128],
            )

        # The first ~22us is DMA-bus-bound: everything before B(0) totals
        # ~7MB at ~360B/ns. Interleave x t-blocks, wv, and per-ob wqk slices
        # so each PE work item's input lands just before PE reaches it.
        # Heads 0,1 need only wqk slices ob0 (q) and ob2 (k).
        fetch_x(0, split=True)
        fetch_x(1)
        fetch_x(2)
        fetch_x(3)
        # wv in two halves at the same queue position: the V projection's
        # first four accumulation steps start on the first half
        wv_r = wv.rearrange("(cb p) n -> p cb n", p=128)
        nc.sync.dma_start(wv_sb[:, 0:4, :], wv_r[:, 0:4, :])
        nc.sync.dma_start(wv_sb[:, 4:CB, :], wv_r[:, 4:CB, :])
        fetch_wqk(0)
        fetch_wqk(2)
        fetch_x(4)
        fetch_wqk(1)
        fetch_wqk(3)
        fetch_x(5)

        def do_T(tb):
            """Transpose one x t-block into xT (bf16 in, f32r out on evac).

            PSUM cells are 32-bit on TRN2 even for bf16 data, so a bank
            holds 512 elements per partition: 4 transposes per PSUM tile."""
            xs = xs_tiles.pop(tb)
            for half in range(2):
                pt = ps_acc.tile([128, 512], BF16, tag="acc", name=f"pt{tb}_{half}")
                for k in range(4):
                    cb = 4 * half + k
                    nc.tensor.transpose(
                        pt[:, k * 128 : (k + 1) * 128],
                        xs[:, cb * 128 : (cb + 1) * 128],
                        ident,
                    )
                nc.vector.tensor_copy(
                    xT[:, 4 * half : 4 * half + 4, tb * 128 : (tb + 1) * 128],
                    pt.rearrange("p (c t) -> p c t", c=4),
                )
            if 6 <= tb + 5 < TB:
                fetch_x(tb + 5)
            if tb == 4:
                # wo is not needed until E(0) (~45us in); keep it off the
                # critical early x/wqk DMA window
                nc.sync.dma_start(wo_sb, wo.rearrange("(cb p) n -> p cb n", p=128))

        def do_V(tb):
            """Project one t-block's V rows (natural layout)."""
            pv = ps_acc.tile([128, 512], F32, tag="acc", name=f"pv{tb}")
            for cb in range(CB):
                nc.tensor.matmul(
                    pv[:, 0:HC],
                    xT[:, cb, tb * 128 : (tb + 1) * 128],
                    wv_sb[:, cb, :],
                    start=(cb == 0),
                    stop=(cb == CB - 1),
                )
            nc.vector.tensor_copy(
                v_sb[:, tb, :, 0:HEAD_DIM],
                pv[:, 0:HC].rearrange("p (h d) -> p h d", h=4),
            )

        def do_tb(tb):
            do_T(tb)
            do_V(tb)

        def do_B_ob(qc, ob):
            """One 128-channel block of the Qt/Kt projection for chunk qc."""
            pq = ps_acc.tile([128, 512], F32, tag="acc", name=f"pq{qc}_{ob}")
            for cb in range(CB):
                nc.tensor.matmul(
                    pq,
                    wqk_sb[:, cb, ob * 128 : (ob + 1) * 128],
                    xT[:, cb, qc * 512 : (qc + 1) * 512],
                    start=(cb == 0),
                    stop=(cb == CB - 1),
                )
            nc.vector.tensor_copy(qkT[:, ob, qc * 512 : (qc + 1) * 512], pq)

        def tail(h, qc, po):
            # normalize: rows 0..63 attn, row 64 softmax denominators
            hp = (h % 2) * 64
            rf = npool.tile([65, 512], F32R, tag="rf", bufs=1)
            with nc.allow_low_precision(
                reason="softmax denominators round to fp32r for the "
                "normalize broadcast; ~1e-4 relative, within tolerance"
            ):
                nc.vector.reciprocal(rf[64:65, :], po[64:65, :])
            # broadcast the reciprocal row across partitions with a K=1
            # PE matmul against a ones column (engines cannot read with
            # partition stride 0; gpsimd partition_broadcast fails codegen)
            pb = ps_acc.tile([128, 512], F32, tag="acc", name=f"pb{h}_{qc}")
            nc.tensor.matmul(
                pb[0:64, :], ones_sb[64:65, :], rf[64:65, :], start=True, stop=True
            )
            bc = npool.tile([64, 512], F32R, tag="bc", bufs=1)
            nc.vector.tensor_copy(bc, pb[0:64, :])
            if hp == 0:
                nc.vector.tensor_mul(
                    ao_sb[0:64, h // 2, qc * 512 : (qc + 1) * 512],
                    po[0:64, :],
                    bc,
                )
            else:
                aos = npool.tile([64, 512], F32R, tag="aos", bufs=1)
                nc.vector.tensor_mul(aos, po[0:64, :], bc)
                # engines cannot shift partitions; DMA moves 0..63->64..127
                nc.sync.dma_start(
                    ao_sb[64:128, h // 2, qc * 512 : (qc + 1) * 512], aos
                )

        pending = None  # deferred normalize: issued after the NEXT job's
        # matmuls so the PE queue never stalls on the reciprocal chain
        pending_avs = []  # the last AHEAD AV matmuls of a job are issued at
        # the START of the next job, so the inter-job filler work (T/V/B/E)
        # runs during the final exp->AV latency instead of PE stalling

        AHEAD = 4  # scores run this many blocks ahead of the AV consumers so
        # the in-order PE queue never ping-pongs with the Act exp latency

        def flush_avs():
            for fn in pending_avs:
                fn()
            pending_avs.clear()

        def do_job(h, qc):
            nonlocal pending
            flush_avs()
            hp = (h % 2) * 64
            qt = qkT[hp : hp + 64, h // 2, :]
            kt = qkT[hp : hp + 64, 2 + h // 2, :]
            po = ps_o.tile([65, 512], F32, tag="po", name=f"po{h}_{qc}")
            nblocks = 4 * (qc + 1)
            avq = []  # (i, off) AV matmuls not yet issued

            def issue_av(i, off):
                p, pc = ppats[i]
                nc.tensor.matmul(
                    po[:, off:512],
                    v_sb[:, i, h, :],
                    p[:, pc : pc + 512 - off],
                    start=(i == 0),
                    stop=(i == nblocks - 1),
                )

            def diag_select(p, pc):
                # zero above-diagonal within the leading 128-wide square of
                # the block slice starting at column pc
                nc.gpsimd.affine_select(
                    out=p[:, pc : pc + 128],
                    in_=p[:, pc : pc + 128],
                    compare_op=mybir.AluOpType.is_ge,
                    fill=0.0,
                    base=0,
                    pattern=[[1, 128]],
                    channel_multiplier=-1,
                )

            ppats = {}
            for i in range(nblocks - 1):
                r = i - 4 * qc  # >=0 on diagonal blocks
                # v/p are bf16, so the AV matmul runs 1cy/row at any moving
                # width (no fp32r N<256 cliff): diagonal blocks shrink to
                # their true causal width
                off = 0 if r < 0 else 128 * r
                w = 512 - off
                last_pair = i == nblocks - 2  # (r2, r3) share one bank + exp
                ps = ps_s.tile([128, 512], F32, tag="ps", name=f"ps{h}_{qc}_{i}")
                nc.tensor.matmul(
                    ps[:, 0:w],
                    kt[:, i * 128 : (i + 1) * 128],
                    qt[:, qc * 512 + off : (qc + 1) * 512],
                    start=True,
                    stop=True,
                )
                p = ppool.tile([128, 512], BF16, tag="p", name=f"p{h}_{qc}_{i}")
                ppats[i] = (p, 0)
                if last_pair:
                    # r3 scores (width 128) pack right after r2's in the
                    # same PSUM bank; one exp covers both
                    nc.tensor.matmul(
                        ps[:, 256:384],
                        kt[:, (i + 1) * 128 : (i + 2) * 128],
                        qt[:, qc * 512 + 384 : (qc + 1) * 512],
                        start=True,
                        stop=True,
                    )
                    ppats[i + 1] = (p, 256)
                    nc.scalar.activation(
                        p[:, 0:384],
                        ps[:, 0:384],
                        mybir.ActivationFunctionType.Exp,
                        scale=scale,
                    )
                    diag_select(p, 0)
                    diag_select(p, 256)
                    avq.append((i, off))
                    avq.append((i + 1, 384))
                else:
                    nc.scalar.activation(
                        p[:, 0:w],
                        ps[:, 0:w],
                        mybir.ActivationFunctionType.Exp,
                        scale=scale,
                    )
                    if r >= 0:
                        diag_select(p, 0)
                    avq.append((i, off))
                if i >= AHEAD:
                    issue_av(*avq.pop(0))
            # the last AHEAD AVs wait on the exp chain; defer them past the
            # inter-job filler work (flushed at the next job's start)
            for a in avq:
                pending_avs.append(lambda a=a: issue_av(*a))
            if pending is not None:
                tail(*pending)
            pending = (h, qc, po)

        def do_E_ob(qc, ob, pool=None, tag="ps", evac=None):
            """One 128-row block of the out-projection for chunk qc."""
            pool = pool or ps_s
            py = pool.tile([128, 512], F32, tag=tag, name=f"py{qc}_{ob}")
            for cb in range(2):
                nc.tensor.matmul(
                    py,
                    wo_sb[:, cb, ob * 128 : (ob + 1) * 128],
                    ao_sb[:, cb, qc * 512 : (qc + 1) * 512],
                    start=(cb == 0),
                    stop=(cb == 1),
                )
            ys = ypool.tile([128, 512], BF16, tag="ys", name=f"ys{qc}_{ob}")
            (evac or nc.vector.tensor_copy)(ys, py)
            nc.sync.dma_start(
                yt[ob * 128 : (ob + 1) * 128, qc * 512 : (qc + 1) * 512], ys
            )

        # ---- the schedule: one interleaved PE stream, no phase barriers ----
        # Front section paced by DMA arrivals: x t-block transposes and V
        # projections as x lands, B(0) ob-slices as their wqk slices land,
        # and D(0) heads 0/1 as soon as ob0+ob2 are projected.
        do_T(0)
        do_T(1)
        do_T(2)
        do_T(3)
        do_V(0)
        do_V(1)
        do_V(2)
        do_V(3)
        do_B_ob(0, 0)
        do_B_ob(0, 2)
        do_T(4)
        do_B_ob(0, 1)
        do_B_ob(0, 3)
        do_V(4)
        do_job(0, 0)
        do_T(5)
        do_job(1, 0)
        do_V(5)
        do_T(6)
        do_job(2, 0)
        do_V(6)
        do_T(7)
        do_job(3, 0)
        do_V(7)
        do_B_ob(1, 0)
        do_B_ob(1, 2)
        # D(1) with B(1) tail, G2 and E(0) fillers.  T(tb) and V(tb) are
        # always separated by other PE work: V waits on the xT evacuation
        # copy (~1.3us after the transposes), so back-to-back T+V stalls.
        do_job(0, 1)
        do_B_ob(1, 1)
        do_B_ob(1, 3)
        do_job(1, 1)
        do_T(8)
        do_E_ob(0, 0)
        do_E_ob(0, 1)
        do_job(2, 1)
        do_T(9)
        do_V(8)
        do_E_ob(0, 2)
        do_job(3, 1)
        do_T(10)
        do_V(9)
        do_E_ob(0, 3)
        do_E_ob(0, 4)
        # D(2) with G3, B(2) and E(0)/E(1) fillers.  Ordering constraints:
        # job(h,2) needs V(0..11) and B(2, qt/kt obs for its head pair.
        do_T(11)
        do_V(10)
        do_B_ob(2, 0)
        do_B_ob(2, 2)
        do_V(11)
        do_job(0, 2)
        do_B_ob(2, 1)
        do_B_ob(2, 3)
        do_E_ob(0, 5)
        do_job(1, 2)
        do_T(12)
        do_E_ob(0, 6)
        do_E_ob(0, 7)
        do_job(2, 2)
        do_T(13)
        do_V(12)
        do_E_ob(1, 0)
        do_job(3, 2)
        do_T(14)
        do_V(13)
        do_E_ob(1, 1)
        do_T(15)
        do_E_ob(1, 2)
        do_V(14)
        do_E_ob(1, 3)
        do_V(15)
        # D(3): odd heads first — the final job's tail must not need the
        # ao partition-shift DMA (it would sit on the critical path into
        # E(3)).  B(3) ob1/ob3 (only needed by heads 2,3) and the E(1) tail
        # chunks are pushed into D(3) as fillers: D(3) jobs have the largest
        # Act-vs-PE deficit (the per-exp access overhead scales with nblocks).
        # E evacs inside D(3) stay off the Activation engine (exp-saturated).
        do_B_ob(3, 0)
        do_B_ob(3, 2)
        do_job(1, 3)
        do_B_ob(3, 1)
        do_B_ob(3, 3)
        do_E_ob(1, 6)
        do_job(3, 3)
        do_E_ob(1, 7)
        do_E_ob(2, 0, pool=ps_acc, tag="acc")
        do_E_ob(2, 1)
        do_E_ob(2, 2, pool=ps_acc, tag="acc")
        do_job(0, 3)
        do_E_ob(1, 4)
        do_E_ob(2, 3)
        do_E_ob(2, 4, pool=ps_acc, tag="acc")
        do_E_ob(2, 5)
        do_job(2, 3)
        do_E_ob(1, 5)
        do_E_ob(2, 6, pool=ps_acc, tag="acc")
        do_E_ob(2, 7)
        # endgame: the final job's normalize is split into column halves so
        # the first E(3) wave starts while the second half normalizes; E(3)
        # chunks rotate across both PSUM rings and both evac engines
        flush_avs()
        fh, fqc, fpo = pending
        pending = None
        rf = npool.tile([65, 512], F32R, tag="rf", bufs=1, name="rf_fin")
        ysf = [
            ypool.tile([128, 512], BF16, tag="ysf", bufs=8, name=f"ysf{ob}")
            for ob in range(CB)
        ]
        for wave, (c0, c1) in enumerate(((0, 256), (256, 512))):
            with nc.allow_low_precision(reason="fp32r softmax denominators"):
                nc.vector.reciprocal(rf[64:65, c0:c1], fpo[64:65, c0:c1])
            pbf = ps_s.tile([128, 512], F32, tag="ps", name=f"pbf{wave}")
            nc.tensor.matmul(
                pbf[0:64, 0 : c1 - c0],
                ones_sb[64:65, :],
                rf[64:65, c0:c1],
                start=True,
                stop=True,
            )
            bcf = npool.tile([64, 512], F32R, tag="bc", bufs=1, name=f"bcf{wave}")
            nc.vector.tensor_copy(bcf[:, 0 : c1 - c0], pbf[0:64, 0 : c1 - c0])
            nc.vector.tensor_mul(
                ao_sb[0:64, fh // 2, fqc * 512 + c0 : fqc * 512 + c1],
                fpo[0:64, c0:c1],
                bcf[:, 0 : c1 - c0],
            )
            for ob in range(CB):
                pool, tg = (ps_acc, "acc") if ob % 2 == 0 else (ps_s, "ps")
                py = pool.tile([128, 512], F32, tag=tg, name=f"pyf{wave}_{ob}")
                for cb in range(2):
                    nc.tensor.matmul(
                        py[:, 0:256],
                        wo_sb[:, cb, ob * 128 : (ob + 1) * 128],
                        ao_sb[:, cb, fqc * 512 + c0 : fqc * 512 + c1],
                        start=(cb == 0),
                        stop=(cb == 1),
                    )
                (nc.scalar.copy if ob % 2 == 0 else nc.vector.tensor_copy)(
                    ysf[ob][:, c0:c1], py[:, 0:256]
                )
                if wave == 1:
                    # one DMA per ob; the first three take Pool's software-DGE
                    # path (1038ns prep each, serialized on the idle Pool
                    # engine) while the rest drain through HWDGE (625ns each),
                    # so the trailing per-DMA fixed overheads run on two
                    # devices in parallel
                    eng = nc.gpsimd if ob < 3 else nc.sync
                    eng.dma_start(
                        yt[ob * 128 : (ob + 1) * 128,
                           fqc * 512 : (fqc + 1) * 512],
                        ysf[ob],
                    )

    split_multi_waits(nc)
    return nc


_NC_CACHE = None


def kernel(x, W_qkv, W_out):
    global _NC_CACHE
    import ml_dtypes

    x = np.asarray(x, dtype=np.float32).astype(ml_dtypes.bfloat16)
    W_qkv = np.asarray(W_qkv, dtype=np.float32)
    W_out = np.asarray(W_out, dtype=np.float32)

    if _NC_CACHE is None:
        _NC_CACHE = build()
    nc = _NC_CACHE

    in_maps = []
    for core in range(N_CORES):
        b, hg = core // 4, core % 4
        cs = hg * HC
        wq = W_qkv[:, cs : cs + HC]
        wk = W_qkv[:, C + cs : C + cs + HC]
        in_maps.append(
            dict(
                xb=np.ascontiguousarray(x[b]),
                wqk=np.ascontiguousarray(
                    np.concatenate([wq, wk], axis=1)
                ).astype(ml_dtypes.bfloat16),
                wv=np.ascontiguousarray(
                    W_qkv[:, 2 * C + cs : 2 * C + cs + HC]
                ).astype(ml_dtypes.bfloat16),
                wo=np.ascontiguousarray(W_out[cs : cs + HC, :]),
            )
        )

    res = run_bass_kernel_spmd(nc, in_maps, core_ids=list(range(N_CORES)))
    out = np.zeros((B, T, C), dtype=np.float32)
    for core in range(N_CORES):
        out[core // 4] += res.results[core]["yt"].astype(np.float32).T
    return out
